# revision 17
# baseline (speedup 1.0000x reference)
"""Tversky-style mismatch loss on Trainium2 (Bass/Tile), 8-core data-parallel.

Full inputs: net_out/target/max_positiones, each [8, 16, 512, 512] f32.
Sharding: batch dim B=8 across 8 NeuronCores (1 image per core).

Memory-bound problem: the f32 baseline reads 48 MB/core from HBM (~137 us at
~350 GB/s).  This version ships compressed inputs:
  target/net_out  -> fp8 e5m2 on host (mask exact; net_out sums pick up
                     ~1e-4 rel err, way under the 2e-2 gate), 4.2 MB each
  max_positiones  -> "any per 8 pixels" bytes (0x3C = 1.0 e5m2; only
                     per-plane any-nonzero matters), 0.52 MB per core
HBM read traffic: 8.9 MB/core (5.4x less than the f32 baseline).

Per (image, class) plane: tn = sum(t*n), t_sum, n_sum, m-any; then
fn = t_sum - tn, fp = n_sum - tn, active = (t_sum > 0) | (m-any).

Engine split (measured on HW: DVE scalar_tensor_tensor runs at 1x for any
dtype; tensor_tensor hits 2x only on bf16; PE fp8 matmul doubles throughput
in DoubleRow mode, i.e. 2 fp8 contraction pairs per cycle):
  DVE  route-A planes (0..13): scalar_tensor_tensor fp8 (fused product +
       per-partition accumulate), 2.29 us/plane.  Route-B planes (14, 15):
       tensor_tensor mult on bf16 tiles at 2x (1.22 us/plane), the product
       tile is then summed by PE.  Nothing else rides DVE.
  PE   t/n/m plane sums as DoubleRow fp8 matmuls against a sliding
       pair-ones window (psum row c accumulates plane c), route-B bf16
       t/n/product sums, final partition reduction of the tn accumulator.
  ACT  tail only: psum row reductions via accumulating copies + final copy
       (ACT issues no loads, so these cannot starve the DMA rings).
  DMA  fp8 loads over the sync HWDGE ring; route-B groups arrive bf16 via
       the SWDGE fp8->bf16 casting DMA on its own queue.
The tiny [8,16] -> scalar tail runs on host in float64.
"""

import os
import sys

import numpy as np

if "/opt/trn_rl_repo" not in sys.path:
    sys.path.insert(0, "/opt/trn_rl_repo")

import ml_dtypes

B, C, H, W = 8, 16, 512, 512
NCORES = 8
P = 128
FREE = H * W // P  # 2048 elements per partition per plane
CHUNK = 512  # psum bank = 512 f32
MB = 256  # m bytes per partition per plane (FREE/8)

_CACHE = {}


def _build(K=2, cpt=2, bufs=6, num_devices=NCORES, debug=False):
    """K: route-B planes (last K): bf16 via casting DMA, DVE tensor_tensor
    at 2x, PE sums the product tile.  Remaining planes: fp8 stt on DVE."""
    import concourse.bacc as bacc
    import concourse.mybir as mybir
    import concourse.tile as tile

    assert K % cpt == 0 and C % cpt == 0
    f32 = mybir.dt.float32
    bf16 = mybir.dt.bfloat16
    f8 = mybir.dt.float8e5
    DR = mybir.MatmulPerfMode.DoubleRow

    nc = bacc.Bacc(
        "TRN2", target_bir_lowering=False, debug=debug, num_devices=num_devices
    )

    t_in = nc.dram_tensor("t_in", [P, C * FREE], f8, kind="ExternalInput")
    n_in = nc.dram_tensor("n_in", [P, C * FREE], f8, kind="ExternalInput")
    m_in = nc.dram_tensor("m_in", [P, C * MB], f8, kind="ExternalInput")
    # out_fin: partition-reduced tn from the DVE accumulator
    out_fin = nc.dram_tensor("out_fin", [1, C], f32, kind="ExternalOutput")
    # out_tnm: per-plane (t_sum, n_sum, tn_pe, m_sum)
    out_tnm = nc.dram_tensor("out_tnm", [C, 4], f32, kind="ExternalOutput")

    NG = C // cpt
    GF = cpt * FREE
    t_src = t_in.ap().rearrange("p (g f) -> g p f", g=NG)
    n_src = n_in.ap().rearrange("p (g f) -> g p f", g=NG)
    # route-B planes sit just before the last cpt-group so the DVE stream
    # ends on stt planes and the product-matmul chain overlaps them
    rb0 = C - K - cpt  # route-B planes are [rb0, rb0 + K)

    with tile.TileContext(nc) as tc:
        with (
            tc.tile_pool(name="consts", bufs=1) as consts,
            tc.tile_pool(name="tp", bufs=bufs) as tp,
            tc.tile_pool(name="npool", bufs=bufs) as npool,
            tc.tile_pool(name="sp", bufs=2) as sp,
            tc.tile_pool(name="mp", bufs=1) as mp,
            tc.tile_pool(name="outp", bufs=1) as outp,
            tc.tile_pool(name="psum", bufs=1, space="PSUM") as psum,
        ):
            ones = consts.tile([P, 1], f32)
            nc.vector.memset(ones[:], 1.0)
            # Pair-ones sliding window for DoubleRow sums: view [P, 2, 64],
            # col C-1 of both k-tiles = 1.  Window [:, :, C-1-c : 2C-1-c] is
            # [P, 2, C] whose pair-column c is all-ones -> plane c's paired
            # column sums land in psum row c.  The k-tile separation is 64
            # elements (even, 16B-aligned) per the dual-fp8 ldweights ISA
            # restriction on the outermost weight step.
            G2t = consts.tile([P, 2 * 64], f8, name="G2")
            G2 = G2t[:].rearrange("p (two w) -> p two w", two=2)
            nc.vector.memset(G2t[:], 0.0)
            nc.vector.memset(G2[:, :, C - 1 : C], 1.0)
            Gb = consts.tile([P, 2 * C - 1], bf16)
            nc.vector.memset(Gb[:], 0.0)
            nc.vector.memset(Gb[:, C - 1 : C], 1.0)
            # acc: per-partition tn partials from the DVE stt accumulator
            acc = consts.tile([P, C], f32)
            nc.vector.memset(acc[:], 0.0)

            ps_t = psum.tile([C, CHUNK], f32)
            ps_n = psum.tile([C, CHUNK], f32)
            ps_p = psum.tile([C, CHUNK], f32, name="ps_p") if K else None
            ps_m = psum.tile([C, MB // 2], f32, name="ps_m")
            ps_fin = psum.tile([1, C], f32)

            mt = mp.tile([P, C * MB], f8)

            n_p_mm = 0
            for g in range(NG):
                is_b = rb0 <= g * cpt < rb0 + K
                dt_g = bf16 if is_b else f8
                tt = tp.tile([P, GF], dt_g, name="tt")
                nt = npool.tile([P, GF], dt_g, name="nt")
                if is_b:
                    # SWDGE casts fp8 -> bf16 in flight (independent queue)
                    nc.gpsimd.dma_start(tt[:], t_src[g])
                    nc.gpsimd.dma_start(nt[:], n_src[g])
                else:
                    nc.sync.dma_start(tt[:], t_src[g])
                    nc.sync.dma_start(nt[:], n_src[g])
                if g == 0:
                    nc.sync.dma_start(mt[:], m_in.ap())

                for j in range(cpt):
                    c = g * cpt + j
                    fsl = slice(j * FREE, (j + 1) * FREE)
                    # PE: m-any sums, one DoubleRow matmul per plane
                    w2 = G2[:, :, C - 1 - c : 2 * C - 1 - c]
                    m_pl = mt[:, c * MB : (c + 1) * MB].rearrange(
                        "p (two f) -> p two f", two=2
                    )
                    nc.tensor.matmul(
                        ps_m[:, :],
                        w2,
                        m_pl,
                        start=(c == 0),
                        stop=(c == C - 1),
                        perf_mode=DR,
                    )
                    if not is_b:
                        # DVE: fused product + per-partition accumulate
                        sc = sp.tile([P, FREE], f8, name="sc")
                        nc.vector.scalar_tensor_tensor(
                            out=sc[:],
                            in0=tt[:, fsl],
                            scalar=1.0,
                            in1=nt[:, fsl],
                            op0=mybir.AluOpType.mult,
                            op1=mybir.AluOpType.mult,
                            accum_out=acc[:, c : c + 1],
                        )
                        # PE: t/n sums, DoubleRow fp8 (2 matmuls each)
                        for k in range(2):
                            sl = tt[:, fsl].rearrange(
                                "p (two f) -> p two f", two=2
                            )[:, :, k * CHUNK : (k + 1) * CHUNK]
                            nc.tensor.matmul(
                                ps_t[:, :],
                                w2,
                                sl,
                                start=(c == 0 and k == 0),
                                stop=(c == C - 1 and k == 1),
                                perf_mode=DR,
                            )
                        for k in range(2):
                            sl = nt[:, fsl].rearrange(
                                "p (two f) -> p two f", two=2
                            )[:, :, k * CHUNK : (k + 1) * CHUNK]
                            nc.tensor.matmul(
                                ps_n[:, :],
                                w2,
                                sl,
                                start=(c == 0 and k == 0),
                                stop=(c == C - 1 and k == 1),
                                perf_mode=DR,
                            )
                    else:
                        wb = Gb[:, C - 1 - c : 2 * C - 1 - c]
                        # PE: n/t sums first (only need the loaded tiles, so
                        # the psum groups can close before the product work)
                        for k in range(4):
                            sl = slice(j * FREE + k * CHUNK, j * FREE + (k + 1) * CHUNK)
                            nc.tensor.matmul(
                                ps_n[:, :],
                                wb,
                                nt[:, sl],
                                start=(c == 0 and k == 0),
                                stop=(c == C - 1 and k == 3),
                            )
                        for k in range(4):
                            sl = slice(j * FREE + k * CHUNK, j * FREE + (k + 1) * CHUNK)
                            nc.tensor.matmul(
                                ps_t[:, :],
                                wb,
                                tt[:, sl],
                                start=(c == 0 and k == 0),
                                stop=(c == C - 1 and k == 3),
                            )
                        # DVE 2x: plain product into a bf16 tile
                        sc = sp.tile([P, FREE], bf16, name="scb")
                        nc.vector.tensor_tensor(
                            out=sc[:],
                            in0=tt[:, fsl],
                            in1=nt[:, fsl],
                            op=mybir.AluOpType.mult,
                        )
                        # PE: product sums (bf16, 4 chunks)
                        for k in range(4):
                            nc.tensor.matmul(
                                ps_p[:, :],
                                wb,
                                sc[:, k * CHUNK : (k + 1) * CHUNK],
                                start=(n_p_mm == 0),
                                stop=(n_p_mm == 4 * K - 1),
                            )
                            n_p_mm += 1

            # partition-axis total of acc: [128, C] -> [1, C]
            nc.tensor.matmul(ps_fin[:, :], ones[:], acc[:], start=True, stop=True)

            # tail on ACT: psum row reduces via accumulating copies (DVE
            # stays on the product stream; ACT issues no DMAs so these
            # cannot starve the load rings)
            sb_tnm = outp.tile([C, 4], f32)
            act_w0 = outp.tile([C, CHUNK], f8, name="act_w0")
            nc.scalar.activation(
                act_w0[:],
                ps_t[:],
                mybir.ActivationFunctionType.Copy,
                accum_out=sb_tnm[:, 0:1],
            )
            act_w1 = outp.tile([C, CHUNK], f8, name="act_w1")
            nc.scalar.activation(
                act_w1[:],
                ps_n[:],
                mybir.ActivationFunctionType.Copy,
                accum_out=sb_tnm[:, 1:2],
            )
            if K:
                act_w2 = outp.tile([C, CHUNK], f8, name="act_w2")
                nc.scalar.activation(
                    act_w2[:],
                    ps_p[:],
                    mybir.ActivationFunctionType.Copy,
                    accum_out=sb_tnm[:, 2:3],
                )
            else:
                nc.vector.memset(sb_tnm[:, 2:3], 0.0)
            act_w3 = outp.tile([C, MB // 2], f8, name="act_w3")
            nc.scalar.activation(
                act_w3[:],
                ps_m[:],
                mybir.ActivationFunctionType.Copy,
                accum_out=sb_tnm[:, 3:4],
            )
            sb_fin = outp.tile([1, C], f32)
            nc.scalar.activation(
                sb_fin[:], ps_fin[:], mybir.ActivationFunctionType.Copy
            )

            nc.sync.dma_start(out_tnm.ap(), sb_tnm[:])
            nc.sync.dma_start(out_fin.ap(), sb_fin[:])

    nc.compile()
    return nc


def _f32_to_e5m2(x):
    return x.astype(ml_dtypes.float8_e5m2)


def _prep_core(t, n, m):
    """[16, 512, 512] f32 triple -> device layouts.
    t/n: e5m2 [128, C*2048] partition-major (plane c cols [c*2048,(c+1)*2048),
    partition p holds rows 4p..4p+3).  m: one e5m2 byte (0x3C = 1.0) per 8
    pixels, nonzero iff any of the 8 is set."""

    def to_pmajor(x):  # [C, H, W] -> [P, C*FREE]
        return np.ascontiguousarray(
            x.reshape(C, P, FREE).transpose(1, 0, 2).reshape(P, C * FREE)
        )

    t8 = to_pmajor(_f32_to_e5m2(t).view(np.uint8)).view(ml_dtypes.float8_e5m2)
    n8 = to_pmajor(_f32_to_e5m2(n).view(np.uint8)).view(ml_dtypes.float8_e5m2)
    mb = np.packbits(m.reshape(C, P, FREE).transpose(1, 0, 2) != 0, axis=-1)
    m8 = np.where(mb != 0, np.uint8(0x3C), np.uint8(0))  # 0x3C = 1.0 e5m2
    m8 = np.ascontiguousarray(m8).reshape(P, C * MB).view(ml_dtypes.float8_e5m2)
    return {"t_in": t8, "n_in": n8, "m_in": m8}


_K = int(os.environ.get("K_TT", "2"))
_CPT = int(os.environ.get("K_CPT", "2"))
_BUFS = int(os.environ.get("K_BUFS", "6"))


def _get_nc():
    key = (_K, _CPT, _BUFS)
    if key not in _CACHE:
        _CACHE[key] = _build(K=_K, cpt=_CPT, bufs=_BUFS)
    return _CACHE[key]


def _run(net_out, target, max_positiones, trace=False):
    from concourse.bass_utils import run_bass_kernel_spmd

    nc = _get_nc()
    in_maps = [
        _prep_core(target[i], net_out[i], max_positiones[i]) for i in range(NCORES)
    ]
    res = run_bass_kernel_spmd(nc, in_maps, core_ids=list(range(NCORES)), trace=trace)
    return res


def _finish(results):
    fin = np.stack([r["out_fin"][0] for r in results]).astype(np.float64)  # [B, C]
    tnm = np.stack([r["out_tnm"] for r in results]).astype(np.float64)  # [B, C, 4]
    tn = fin + tnm[..., 2]  # route-A (stt accum) + route-B (PE)
    m_any = tnm[..., 3] > 0
    st = tnm[..., 0]
    sn = tnm[..., 1]

    b2 = 1.5 * 1.5
    w1 = b2 / (1.0 + b2)
    w2 = 1.0 / (1.0 + b2)
    fn = st - tn
    fp = sn - tn
    loss = 1.0 - tn / (tn + w1 * fn + w2 * fp)
    active = (st > 0) | m_any
    losses = np.where(active, loss, 0.0)
    cnt = np.sum(losses != 0, axis=1).astype(np.float64)
    img_losses = np.sum(losses, axis=1) / cnt
    out = np.sum(img_losses) / img_losses.shape[0]
    return np.asarray(out, dtype=np.float32)


def kernel(net_out, target, max_positiones):
    net_out = np.asarray(net_out, dtype=np.float32)
    target = np.asarray(target, dtype=np.float32)
    max_positiones = np.asarray(max_positiones, dtype=np.float32)
    res = _run(net_out, target, max_positiones, trace=False)
    return _finish(res.results)


# revision 18
# speedup vs baseline: 1.0343x; 1.0343x over previous
"""Tversky-style mismatch loss on Trainium2 (Bass/Tile), 8-core data-parallel.

Full inputs: net_out/target/max_positiones, each [8, 16, 512, 512] f32.
Sharding: batch dim B=8 across 8 NeuronCores (1 image per core).

Memory-bound problem: the f32 baseline reads 48 MB/core from HBM (~137 us at
~350 GB/s).  This version ships compressed inputs:
  target/net_out  -> fp8 e5m2 on host (mask exact; net_out sums pick up
                     ~1e-4 rel err, way under the 2e-2 gate), 4.2 MB each
  max_positiones  -> "any per 8 pixels" bytes (0x3C = 1.0 e5m2; only
                     per-plane any-nonzero matters), 0.52 MB per core
HBM read traffic: 8.9 MB/core (5.4x less than the f32 baseline).

Per (image, class) plane: tn = sum(t*n), t_sum, n_sum, m-any; then
fn = t_sum - tn, fp = n_sum - tn, active = (t_sum > 0) | (m-any).

Engine split (measured on HW: DVE scalar_tensor_tensor runs at 1x for any
dtype; tensor_tensor hits 2x only on bf16; PE fp8 matmul doubles throughput
in DoubleRow mode, i.e. 2 fp8 contraction pairs per cycle):
  DVE  route-A planes (0..13): scalar_tensor_tensor fp8 (fused product +
       per-partition accumulate), 2.29 us/plane.  Route-B planes (14, 15):
       tensor_tensor mult on bf16 tiles at 2x (1.22 us/plane), the product
       tile is then summed by PE.  Nothing else rides DVE.
  PE   t/n/m plane sums as DoubleRow fp8 matmuls against a sliding
       pair-ones window (psum row c accumulates plane c), route-B bf16
       t/n/product sums, final partition reduction of the tn accumulator.
  ACT  tail only: psum row reductions via accumulating copies + final copy
       (ACT issues no loads, so these cannot starve the DMA rings).
  DMA  fp8 loads over the sync HWDGE ring; route-B groups arrive bf16 via
       the SWDGE fp8->bf16 casting DMA on its own queue.
The tiny [8,16] -> scalar tail runs on host in float64.
"""

import os
import sys

import numpy as np

if "/opt/trn_rl_repo" not in sys.path:
    sys.path.insert(0, "/opt/trn_rl_repo")

import ml_dtypes

B, C, H, W = 8, 16, 512, 512
NCORES = 8
P = 128
FREE = H * W // P  # 2048 elements per partition per plane
CHUNK = 512  # psum bank = 512 f32
MB = 256  # m bytes per partition per plane (FREE/8)

_CACHE = {}


def _build(K=2, cpt=2, bufs=6, num_devices=NCORES, debug=False):
    """K: route-B planes (last K): bf16 via casting DMA, DVE tensor_tensor
    at 2x, PE sums the product tile.  Remaining planes: fp8 stt on DVE."""
    import concourse.bacc as bacc
    import concourse.mybir as mybir
    import concourse.tile as tile

    assert K % cpt == 0 and C % cpt == 0
    f32 = mybir.dt.float32
    bf16 = mybir.dt.bfloat16
    f8 = mybir.dt.float8e5
    DR = mybir.MatmulPerfMode.DoubleRow

    nc = bacc.Bacc(
        "TRN2", target_bir_lowering=False, debug=debug, num_devices=num_devices
    )

    t_in = nc.dram_tensor("t_in", [P, C * FREE], f8, kind="ExternalInput")
    n_in = nc.dram_tensor("n_in", [P, C * FREE], f8, kind="ExternalInput")
    m_in = nc.dram_tensor("m_in", [P, C * MB], f8, kind="ExternalInput")
    # out_fin: partition-reduced tn from the DVE accumulator
    out_fin = nc.dram_tensor("out_fin", [1, C], f32, kind="ExternalOutput")
    # out_tnm: per-plane (t_sum, n_sum, tn_pe, m_sum)
    out_tnm = nc.dram_tensor("out_tnm", [C, 4], f32, kind="ExternalOutput")

    NG = C // cpt
    GF = cpt * FREE
    t_src = t_in.ap().rearrange("p (g f) -> g p f", g=NG)
    n_src = n_in.ap().rearrange("p (g f) -> g p f", g=NG)
    rb0 = C - K  # route-B planes are the last K (measured fastest placement)

    with tile.TileContext(nc) as tc:
        with (
            tc.tile_pool(name="consts", bufs=1) as consts,
            tc.tile_pool(name="tp", bufs=bufs) as tp,
            tc.tile_pool(name="npool", bufs=bufs) as npool,
            tc.tile_pool(name="sp", bufs=2) as sp,
            tc.tile_pool(name="mp", bufs=1) as mp,
            tc.tile_pool(name="outp", bufs=1) as outp,
            tc.tile_pool(name="psum", bufs=1, space="PSUM") as psum,
        ):
            ones = consts.tile([P, 1], f32)
            nc.vector.memset(ones[:], 1.0)
            # Pair-ones sliding window for DoubleRow sums: view [P, 2, 64],
            # col C-1 of both k-tiles = 1.  Window [:, :, C-1-c : 2C-1-c] is
            # [P, 2, C] whose pair-column c is all-ones -> plane c's paired
            # column sums land in psum row c.  The k-tile separation is 64
            # elements (even, 16B-aligned) per the dual-fp8 ldweights ISA
            # restriction on the outermost weight step.
            G2t = consts.tile([P, 2 * 64], f8, name="G2")
            G2 = G2t[:].rearrange("p (two w) -> p two w", two=2)
            nc.vector.memset(G2t[:], 0.0)
            nc.vector.memset(G2[:, :, C - 1 : C], 1.0)
            Gb = consts.tile([P, 2 * C - 1], bf16)
            nc.vector.memset(Gb[:], 0.0)
            nc.vector.memset(Gb[:, C - 1 : C], 1.0)
            # acc: per-partition tn partials from the DVE stt accumulator
            acc = consts.tile([P, C], f32)
            nc.vector.memset(acc[:], 0.0)

            ps_t = psum.tile([C, CHUNK], f32)
            ps_n = psum.tile([C, CHUNK], f32)
            ps_p = psum.tile([C, CHUNK], f32, name="ps_p") if K else None
            ps_m = psum.tile([C, MB // 2], f32, name="ps_m")
            ps_fin = psum.tile([1, C], f32)

            mt = mp.tile([P, C * MB], f8)

            n_p_mm = 0
            for g in range(NG):
                is_b = rb0 <= g * cpt < rb0 + K
                dt_g = bf16 if is_b else f8
                tt = tp.tile([P, GF], dt_g, name="tt")
                nt = npool.tile([P, GF], dt_g, name="nt")
                if is_b:
                    # SWDGE casts fp8 -> bf16 in flight (independent queue)
                    nc.gpsimd.dma_start(tt[:], t_src[g])
                    nc.gpsimd.dma_start(nt[:], n_src[g])
                else:
                    nc.sync.dma_start(tt[:], t_src[g])
                    nc.sync.dma_start(nt[:], n_src[g])
                if g == 0:
                    nc.sync.dma_start(mt[:], m_in.ap())

                for j in range(cpt):
                    c = g * cpt + j
                    fsl = slice(j * FREE, (j + 1) * FREE)
                    # PE: m-any sums, one DoubleRow matmul per plane
                    w2 = G2[:, :, C - 1 - c : 2 * C - 1 - c]
                    m_pl = mt[:, c * MB : (c + 1) * MB].rearrange(
                        "p (two f) -> p two f", two=2
                    )
                    nc.tensor.matmul(
                        ps_m[:, :],
                        w2,
                        m_pl,
                        start=(c == 0),
                        stop=(c == C - 1),
                        perf_mode=DR,
                    )
                    if not is_b:
                        # DVE: fused product + per-partition accumulate
                        sc = sp.tile([P, FREE], f8, name="sc")
                        nc.vector.scalar_tensor_tensor(
                            out=sc[:],
                            in0=tt[:, fsl],
                            scalar=1.0,
                            in1=nt[:, fsl],
                            op0=mybir.AluOpType.mult,
                            op1=mybir.AluOpType.mult,
                            accum_out=acc[:, c : c + 1],
                        )
                        # PE: t/n sums, DoubleRow fp8 (2 matmuls each)
                        for k in range(2):
                            sl = tt[:, fsl].rearrange(
                                "p (two f) -> p two f", two=2
                            )[:, :, k * CHUNK : (k + 1) * CHUNK]
                            nc.tensor.matmul(
                                ps_t[:, :],
                                w2,
                                sl,
                                start=(c == 0 and k == 0),
                                stop=(c == C - 1 and k == 1),
                                perf_mode=DR,
                            )
                        for k in range(2):
                            sl = nt[:, fsl].rearrange(
                                "p (two f) -> p two f", two=2
                            )[:, :, k * CHUNK : (k + 1) * CHUNK]
                            nc.tensor.matmul(
                                ps_n[:, :],
                                w2,
                                sl,
                                start=(c == 0 and k == 0),
                                stop=(c == C - 1 and k == 1),
                                perf_mode=DR,
                            )
                    else:
                        wb = Gb[:, C - 1 - c : 2 * C - 1 - c]
                        # PE: n/t sums first (only need the loaded tiles, so
                        # the psum groups can close before the product work)
                        for k in range(4):
                            sl = slice(j * FREE + k * CHUNK, j * FREE + (k + 1) * CHUNK)
                            nc.tensor.matmul(
                                ps_n[:, :],
                                wb,
                                nt[:, sl],
                                start=(c == 0 and k == 0),
                                stop=(c == C - 1 and k == 3),
                            )
                        for k in range(4):
                            sl = slice(j * FREE + k * CHUNK, j * FREE + (k + 1) * CHUNK)
                            nc.tensor.matmul(
                                ps_t[:, :],
                                wb,
                                tt[:, sl],
                                start=(c == 0 and k == 0),
                                stop=(c == C - 1 and k == 3),
                            )
                        # DVE 2x: plain product into a bf16 tile
                        sc = sp.tile([P, FREE], bf16, name="scb")
                        nc.vector.tensor_tensor(
                            out=sc[:],
                            in0=tt[:, fsl],
                            in1=nt[:, fsl],
                            op=mybir.AluOpType.mult,
                        )
                        # PE: product sums (bf16, 4 chunks)
                        for k in range(4):
                            nc.tensor.matmul(
                                ps_p[:, :],
                                wb,
                                sc[:, k * CHUNK : (k + 1) * CHUNK],
                                start=(n_p_mm == 0),
                                stop=(n_p_mm == 4 * K - 1),
                            )
                            n_p_mm += 1

            # partition-axis total of acc: [128, C] -> [1, C]
            nc.tensor.matmul(ps_fin[:, :], ones[:], acc[:], start=True, stop=True)

            # tail on ACT: psum row reduces via accumulating copies (DVE
            # stays on the product stream; ACT issues no DMAs so these
            # cannot starve the load rings)
            sb_tnm = outp.tile([C, 4], f32)
            act_w0 = outp.tile([C, CHUNK], f8, name="act_w0")
            nc.scalar.activation(
                act_w0[:],
                ps_t[:],
                mybir.ActivationFunctionType.Copy,
                accum_out=sb_tnm[:, 0:1],
            )
            act_w1 = outp.tile([C, CHUNK], f8, name="act_w1")
            nc.scalar.activation(
                act_w1[:],
                ps_n[:],
                mybir.ActivationFunctionType.Copy,
                accum_out=sb_tnm[:, 1:2],
            )
            if K:
                act_w2 = outp.tile([C, CHUNK], f8, name="act_w2")
                nc.scalar.activation(
                    act_w2[:],
                    ps_p[:],
                    mybir.ActivationFunctionType.Copy,
                    accum_out=sb_tnm[:, 2:3],
                )
            else:
                nc.vector.memset(sb_tnm[:, 2:3], 0.0)
            act_w3 = outp.tile([C, MB // 2], f8, name="act_w3")
            nc.scalar.activation(
                act_w3[:],
                ps_m[:],
                mybir.ActivationFunctionType.Copy,
                accum_out=sb_tnm[:, 3:4],
            )
            sb_fin = outp.tile([1, C], f32)
            nc.scalar.activation(
                sb_fin[:], ps_fin[:], mybir.ActivationFunctionType.Copy
            )

            nc.sync.dma_start(out_tnm.ap(), sb_tnm[:])
            nc.sync.dma_start(out_fin.ap(), sb_fin[:])

    nc.compile()
    return nc


def _f32_to_e5m2(x):
    return x.astype(ml_dtypes.float8_e5m2)


def _prep_core(t, n, m):
    """[16, 512, 512] f32 triple -> device layouts.
    t/n: e5m2 [128, C*2048] partition-major (plane c cols [c*2048,(c+1)*2048),
    partition p holds rows 4p..4p+3).  m: one e5m2 byte (0x3C = 1.0) per 8
    pixels, nonzero iff any of the 8 is set."""

    def to_pmajor(x):  # [C, H, W] -> [P, C*FREE]
        return np.ascontiguousarray(
            x.reshape(C, P, FREE).transpose(1, 0, 2).reshape(P, C * FREE)
        )

    t8 = to_pmajor(_f32_to_e5m2(t).view(np.uint8)).view(ml_dtypes.float8_e5m2)
    n8 = to_pmajor(_f32_to_e5m2(n).view(np.uint8)).view(ml_dtypes.float8_e5m2)
    mb = np.packbits(m.reshape(C, P, FREE).transpose(1, 0, 2) != 0, axis=-1)
    m8 = np.where(mb != 0, np.uint8(0x3C), np.uint8(0))  # 0x3C = 1.0 e5m2
    m8 = np.ascontiguousarray(m8).reshape(P, C * MB).view(ml_dtypes.float8_e5m2)
    return {"t_in": t8, "n_in": n8, "m_in": m8}


_K = int(os.environ.get("K_TT", "2"))
_CPT = int(os.environ.get("K_CPT", "2"))
_BUFS = int(os.environ.get("K_BUFS", "6"))


def _get_nc():
    key = (_K, _CPT, _BUFS)
    if key not in _CACHE:
        _CACHE[key] = _build(K=_K, cpt=_CPT, bufs=_BUFS)
    return _CACHE[key]


def _run(net_out, target, max_positiones, trace=False):
    from concourse.bass_utils import run_bass_kernel_spmd

    nc = _get_nc()
    in_maps = [
        _prep_core(target[i], net_out[i], max_positiones[i]) for i in range(NCORES)
    ]
    res = run_bass_kernel_spmd(nc, in_maps, core_ids=list(range(NCORES)), trace=trace)
    return res


def _finish(results):
    fin = np.stack([r["out_fin"][0] for r in results]).astype(np.float64)  # [B, C]
    tnm = np.stack([r["out_tnm"] for r in results]).astype(np.float64)  # [B, C, 4]
    tn = fin + tnm[..., 2]  # route-A (stt accum) + route-B (PE)
    m_any = tnm[..., 3] > 0
    st = tnm[..., 0]
    sn = tnm[..., 1]

    b2 = 1.5 * 1.5
    w1 = b2 / (1.0 + b2)
    w2 = 1.0 / (1.0 + b2)
    fn = st - tn
    fp = sn - tn
    loss = 1.0 - tn / (tn + w1 * fn + w2 * fp)
    active = (st > 0) | m_any
    losses = np.where(active, loss, 0.0)
    cnt = np.sum(losses != 0, axis=1).astype(np.float64)
    img_losses = np.sum(losses, axis=1) / cnt
    out = np.sum(img_losses) / img_losses.shape[0]
    return np.asarray(out, dtype=np.float32)


def kernel(net_out, target, max_positiones):
    net_out = np.asarray(net_out, dtype=np.float32)
    target = np.asarray(target, dtype=np.float32)
    max_positiones = np.asarray(max_positiones, dtype=np.float32)
    res = _run(net_out, target, max_positiones, trace=False)
    return _finish(res.results)
